# revision 6
# baseline (speedup 1.0000x reference)
"""Multi-head causal attention (B=2, T=2048, D=1024, H=16) on 8 trn2 cores.

Sharding: core = (b, g) with b = batch (2), g = head-group of 4 heads (4).
Each core:
  phase 1: qkv projection for its 4 heads  (qT/kT feature-major, v token-major)
  phase 2: causal attention, scores computed transposed (S^T[j, i]) so the
           softmax denominator comes out of the PV matmul via an appended
           ones-column on V, and no on-device transposes are needed
  phase 3: partial output projection y^T = wo_slice^T @ attn^T
Host: sums the 4 per-batch partials, adds out_b + out_w @ v_bias (v-bias
commutes through the normalized softmax), k-bias dropped entirely (softmax
row-shift invariance), q-bias applied on device.
"""

import sys

if "/opt/trn_rl_repo" not in sys.path:
    sys.path.insert(0, "/opt/trn_rl_repo")

import numpy as np

D_MODEL = 1024
N_HEADS = 16
HEAD_DIM = 64
N_CORES = 8
N_GROUPS = 4                     # head groups (tensor parallel)
HEADS = N_HEADS // N_GROUPS      # heads per core = 4
JQ = HEADS * HEAD_DIM            # per-core q/k/v features = 256
IB = 512                         # i-block (query block) width
JB = 128                         # j-chunk (key block) width


def _split_sync_waits(nc, max_waits=1):
    """walrus in this env rejects >1 sync wait on an instruction; move
    extras onto preceding same-engine NOPs."""
    import bass_rust
    import concourse.mybir as mybir

    uid = 0
    for f in nc.m.functions:
        for bb in f.blocks:
            insts = list(bb.instructions)
            out = []
            changed = False
            for ins in insts:
                si = ins.sync_info
                waits = list(si.on_wait) if si is not None else []
                if len(waits) > max_waits:
                    changed = True
                    extras, keep = waits[:-max_waits], waits[-max_waits:]
                    for i in range(0, len(extras), max_waits):
                        grp = extras[i:i + max_waits]
                        nop = mybir.InstNoOp(name=f"WS-{uid}", ins=[], outs=[])
                        uid += 1
                        nop.engine = ins.engine
                        nop.sync_info = bass_rust.SyncInfo(on_wait=grp, on_update=[])
                        out.append(nop)
                    ins.sync_info = bass_rust.SyncInfo(
                        on_wait=keep, on_update=list(si.on_update))
                out.append(ins)
            if changed:
                bb.instructions = out
    return nc


_PROG_CACHE = {}


def _build_program(T, repeat=1):
    import concourse.bass as bass
    import concourse.mybir as mybir
    import concourse.tile as tile
    from concourse.bass import ts

    f32 = mybir.dt.float32
    f32r = mybir.dt.float32r
    D = D_MODEL
    HD = HEAD_DIM
    DC = D // 128                 # contraction chunks (8)
    JCH = JQ // 128               # q/k feature chunks (2)
    TC = T // IB                  # 512-wide token chunks
    NB = T // JB                  # 128-wide token chunks
    GB = T // IB                  # i-blocks
    VW = HD + 1                   # v + ones column

    nc = bass.Bass(target_bir_lowering=False)

    xT_d = nc.dram_tensor("xT", [D, T], f32r, kind="ExternalInput")
    wq_d = nc.dram_tensor("wq", [D, JQ], f32r, kind="ExternalInput")
    wk_d = nc.dram_tensor("wk", [D, JQ], f32r, kind="ExternalInput")
    wv_d = nc.dram_tensor("wv", [D, JQ], f32r, kind="ExternalInput")
    wo_d = nc.dram_tensor("wo", [JQ, D], f32r, kind="ExternalInput")
    bq_d = nc.dram_tensor("bq", [128, JCH], f32, kind="ExternalInput")
    mk_d = nc.dram_tensor("mk", [128, 4 * IB], f32, kind="ExternalInput")
    yT_d = nc.dram_tensor("yT", [D, T], f32, kind="ExternalOutput")

    def mm(out, lhsT, rhs, start, stop):
        nc.tensor.matmul(out, lhsT, rhs, start=start, stop=stop)

    with tile.TileContext(nc) as tc:
        with tc.tile_pool(name="persist", bufs=1) as pp:
            wq_s = pp.tile([128, DC, JQ], f32r, tag="wq")
            wk_s = pp.tile([128, DC, JQ], f32r, tag="wk")
            wv_s = pp.tile([128, DC, JQ], f32r, tag="wv")
            wo_s = pp.tile([128, JCH, D], f32r, tag="wo")
            bq_s = pp.tile([128, JCH], f32, tag="bq")
            mk_s = pp.tile([128, 4, IB], f32, tag="mk")
            ones_s = pp.tile([1, 64], f32r, tag="ones")
            ones32_s = pp.tile([128, 64], f32, tag="ones32")
            qT_s = pp.tile([128, JCH, T], f32r, tag="qT")
            kT_s = pp.tile([128, JCH, T], f32r, tag="kT")
            v_s = pp.tile([128, NB, HEADS * VW], f32r, tag="v")
            on_s = pp.tile([128, JCH, T], f32r, tag="otn")

            nc.sync.dma_start(wq_s[:], wq_d.rearrange("(c p) j -> p c j", p=128))
            nc.sync.dma_start(wk_s[:], wk_d.rearrange("(c p) j -> p c j", p=128))
            nc.sync.dma_start(wv_s[:], wv_d.rearrange("(c p) j -> p c j", p=128))
            nc.sync.dma_start(wo_s[:], wo_d.rearrange("(c p) e -> p c e", p=128))
            nc.sync.dma_start(bq_s[:], bq_d[:])
            nc.sync.dma_start(mk_s[:], mk_d.rearrange("p (k i) -> p k i", k=4))
            nc.vector.memset(ones32_s[:], 1.0)
            nc.vector.tensor_copy(ones_s[:], ones32_s[0:1, :])

            for _rep in range(repeat):
                # ---------------- phase 1: qkv projection ----------------
                with tc.tile_pool(name="xs", bufs=2) as xp, \
                     tc.tile_pool(name="ps1", bufs=6, space="PSUM") as ps1:
                    for tci in range(TC):
                        x_t = xp.tile([128, DC, IB], f32r, tag="x")
                        nc.sync.dma_start(
                            x_t[:],
                            xT_d[:, ts(tci, IB)].rearrange("(c p) t -> p c t", p=128))
                        for jc in range(JCH):
                            q_ps = ps1.tile([128, IB], f32, tag="ps1")
                            for dc in range(DC):
                                mm(q_ps[:], wq_s[:, dc, ts(jc, 128)], x_t[:, dc, :],
                                   dc == 0, dc == DC - 1)
                            nc.vector.tensor_scalar_add(
                                qT_s[:, jc, ts(tci, IB)], q_ps[:], bq_s[:, jc:jc + 1])
                        for jc in range(JCH):
                            k_ps = ps1.tile([128, IB], f32, tag="ps1")
                            for dc in range(DC):
                                mm(k_ps[:], wk_s[:, dc, ts(jc, 128)], x_t[:, dc, :],
                                   dc == 0, dc == DC - 1)
                            nc.vector.tensor_copy(kT_s[:, jc, ts(tci, IB)], k_ps[:])
                        for s in range(IB // JB):
                            v_ps = ps1.tile([128, IB], f32, tag="ps1")
                            for dc in range(DC):
                                mm(v_ps[:, :JQ], x_t[:, dc, ts(s, 128)], wv_s[:, dc, :],
                                   dc == 0, dc == DC - 1)
                            bi = tci * (IB // JB) + s
                            vv = v_s[:, bi, :].rearrange("p (h c) -> p h c", c=VW)
                            nc.vector.tensor_copy(
                                vv[:, :, 0:HD],
                                v_ps[:, :JQ].rearrange("p (h c) -> p h c", c=HD))
                            nc.vector.tensor_copy(
                                vv[:, :, HD:VW],
                                ones32_s[:, 0:HEADS].rearrange(
                                    "p (a b) -> p a b", b=1))

                # ---------------- phase 2: causal attention ----------------
                with tc.tile_pool(name="pt", bufs=6) as ptp, \
                     tc.tile_pool(name="rsb", bufs=2) as rsb, \
                     tc.tile_pool(name="pss", bufs=4, space="PSUM") as pss, \
                     tc.tile_pool(name="pso", bufs=2, space="PSUM") as pso, \
                     tc.tile_pool(name="psr", bufs=2, space="PSUM") as psr:
                    for h in range(HEADS):
                        jc = h // 2
                        p0 = 64 * (h % 2)
                        for g in range(GB):
                            ncj = (g + 1) * (IB // JB)
                            o_ps = pso.tile([HD + 1, IB], f32, tag="ot")
                            for cj in range(ncj):
                                s_ps = pss.tile([128, IB], f32, tag="s")
                                mm(s_ps[:],
                                   kT_s[p0:p0 + HD, jc, ts(cj, JB)],
                                   qT_s[p0:p0 + HD, jc, ts(g, IB)],
                                   True, True)
                                p_t = ptp.tile([128, IB], f32r, tag="p")
                                nc.scalar.activation(
                                    p_t[:], s_ps[:],
                                    mybir.ActivationFunctionType.Exp, scale=0.125)
                                kd = cj - g * (IB // JB)
                                if kd >= 0:
                                    nc.vector.tensor_mul(p_t[:], p_t[:], mk_s[:, kd, :])
                                mm(o_ps[:],
                                   v_s[:, cj, :].rearrange(
                                       "p (h c) -> p h c", c=VW)[:, h, :],
                                   p_t[:], cj == 0, cj == ncj - 1)
                            # normalize columns by 1/l (row HD of o_ps)
                            r_t = rsb.tile([1, IB], f32r, tag="r")
                            with nc.allow_low_precision(reason="1/l in f32r feeds PE broadcast"):
                                nc.vector.reciprocal(r_t[:], o_ps[HD:HD + 1, :])
                            rb_ps = psr.tile([64, IB], f32, tag="rb")
                            mm(rb_ps[:], ones_s[:], r_t[:], True, True)
                            rb_t = rsb.tile([64, IB], f32, tag="rbs")
                            nc.vector.tensor_copy(rb_t[:], rb_ps[:])
                            nc.vector.tensor_mul(
                                on_s[p0:p0 + HD, jc, ts(g, IB)],
                                o_ps[0:HD, :], rb_t[:])

                # ---------------- phase 3: output projection ----------------
                with tc.tile_pool(name="ys", bufs=4) as ysp, \
                     tc.tile_pool(name="ps3", bufs=4, space="PSUM") as ps3:
                    for ec in range(DC):
                        for tci in range(TC):
                            y_ps = ps3.tile([128, IB], f32, tag="y")
                            for cc in range(JCH):
                                mm(y_ps[:], wo_s[:, cc, ts(ec, 128)],
                                   on_s[:, cc, ts(tci, IB)],
                                   cc == 0, cc == JCH - 1)
                            y_sb = ysp.tile([128, IB], f32, tag="ysb")
                            nc.vector.tensor_copy(y_sb[:], y_ps[:])
                            nc.sync.dma_start(
                                yT_d[ts(ec, 128), ts(tci, IB)], y_sb[:])

    _split_sync_waits(nc, max_waits=1)
    return nc


def _get_program(T, repeat=1):
    key = (T, repeat)
    if key not in _PROG_CACHE:
        _PROG_CACHE[key] = _build_program(T, repeat)
    return _PROG_CACHE[key]


def _make_masks():
    ii = np.arange(IB)[None, :]
    jj = np.arange(JB)[:, None]
    return np.concatenate(
        [(ii >= jj + JB * k).astype(np.float32) for k in range(4)],
        axis=1)  # [128, 4*IB]


def _core_inputs(x, qkv_w, qkv_b, out_w, T):
    D = D_MODEL
    masks = _make_masks()
    maps = []
    for core in range(N_CORES):
        b, g = divmod(core, N_GROUPS)
        r0 = JQ * g
        xT = np.ascontiguousarray(x[b].T)                       # [D, T]
        wq = np.ascontiguousarray(qkv_w[r0:r0 + JQ, :].T)       # [D, JQ]
        wk = np.ascontiguousarray(qkv_w[D + r0:D + r0 + JQ, :].T)
        wv = np.ascontiguousarray(qkv_w[2 * D + r0:2 * D + r0 + JQ, :].T)
        wo = np.ascontiguousarray(out_w[:, r0:r0 + JQ].T)       # [JQ, D]
        bq = np.ascontiguousarray(
            qkv_b[r0:r0 + JQ].reshape(JQ // 128, 128).T)        # [128, JCH]
        maps.append({"xT": xT, "wq": wq, "wk": wk, "wv": wv,
                     "wo": wo, "bq": bq, "mk": masks})
    return maps


def _gather(results, out_b, out_w, qkv_b, B, T):
    D = D_MODEL
    y = np.zeros((B, T, D), np.float32)
    for core, om in enumerate(results):
        b = core // N_GROUPS
        y[b] += om["yT"].T
    bv = qkv_b[2 * D:3 * D]
    y += (out_b + out_w @ bv)[None, None, :].astype(np.float32)
    return y


def kernel(x, qkv_w, qkv_b, out_w, out_b):
    from concourse.bass_utils import run_bass_kernel_spmd

    x = np.asarray(x, np.float32)
    B, T, D = x.shape
    assert D == D_MODEL and B * N_GROUPS == N_CORES
    nc = _get_program(T)
    in_maps = _core_inputs(x, qkv_w, qkv_b, out_w, T)
    res = run_bass_kernel_spmd(nc, in_maps, list(range(N_CORES)))
    return _gather(res.results, np.asarray(out_b, np.float32),
                   np.asarray(out_w, np.float32),
                   np.asarray(qkv_b, np.float32), B, T)


# revision 7
# speedup vs baseline: 152.3234x; 152.3234x over previous
"""Multi-head causal attention (B=2, T=2048, D=1024, H=16) on 8 trn2 cores.

Sharding: core = (b, g) with b = batch (2), g = head-group of 4 heads (4).
Each core:
  phase 1: qkv projection for its 4 heads  (qT/kT feature-major, v token-major)
  phase 2: causal attention, scores computed transposed (S^T[j, i]) so the
           softmax denominator comes out of the PV matmul via an appended
           ones-column on V, and no on-device transposes are needed
  phase 3: partial output projection y^T = wo_slice^T @ attn^T
Host: sums the 4 per-batch partials, adds out_b + out_w @ v_bias (v-bias
commutes through the normalized softmax), k-bias dropped entirely (softmax
row-shift invariance), q-bias applied on device.
"""

import sys

if "/opt/trn_rl_repo" not in sys.path:
    sys.path.insert(0, "/opt/trn_rl_repo")

import numpy as np

D_MODEL = 1024
N_HEADS = 16
HEAD_DIM = 64
N_CORES = 8
N_GROUPS = 4                     # head groups (tensor parallel)
HEADS = N_HEADS // N_GROUPS      # heads per core = 4
JQ = HEADS * HEAD_DIM            # per-core q/k/v features = 256
IB = 512                         # i-block (query block) width
JB = 128                         # j-chunk (key block) width


def _split_sync_waits(nc, max_waits=1):
    """walrus in this env rejects >1 sync wait on an instruction; move
    extras onto preceding same-engine NOPs."""
    import bass_rust
    import concourse.mybir as mybir

    uid = 0
    for f in nc.m.functions:
        for bb in f.blocks:
            insts = list(bb.instructions)
            out = []
            changed = False
            for ins in insts:
                si = ins.sync_info
                waits = list(si.on_wait) if si is not None else []
                if len(waits) > max_waits:
                    changed = True
                    extras, keep = waits[:-max_waits], waits[-max_waits:]
                    for i in range(0, len(extras), max_waits):
                        grp = extras[i:i + max_waits]
                        nop = mybir.InstNoOp(name=f"WS-{uid}", ins=[], outs=[])
                        uid += 1
                        nop.engine = ins.engine
                        nop.sync_info = bass_rust.SyncInfo(on_wait=grp, on_update=[])
                        out.append(nop)
                    ins.sync_info = bass_rust.SyncInfo(
                        on_wait=keep, on_update=list(si.on_update))
                out.append(ins)
            if changed:
                bb.instructions = out
    return nc


_PROG_CACHE = {}


def _build_program(T, repeat=1):
    import concourse.bass as bass
    import concourse.mybir as mybir
    import concourse.tile as tile
    from concourse.bass import ts

    f32 = mybir.dt.float32
    f32r = mybir.dt.float32r
    D = D_MODEL
    HD = HEAD_DIM
    DC = D // 128                 # contraction chunks (8)
    JCH = JQ // 128               # q/k feature chunks (2)
    TC = T // IB                  # 512-wide token chunks
    NB = T // JB                  # 128-wide token chunks
    GB = T // IB                  # i-blocks
    VW = HD + 1                   # v + ones column

    nc = bass.Bass(target_bir_lowering=False)

    xT_d = nc.dram_tensor("xT", [D, T], f32r, kind="ExternalInput")
    wq_d = nc.dram_tensor("wq", [D, JQ], f32r, kind="ExternalInput")
    wk_d = nc.dram_tensor("wk", [D, JQ], f32r, kind="ExternalInput")
    wv_d = nc.dram_tensor("wv", [D, JQ], f32r, kind="ExternalInput")
    wo_d = nc.dram_tensor("wo", [JQ, D], f32r, kind="ExternalInput")
    bq_d = nc.dram_tensor("bq", [128, JCH], f32, kind="ExternalInput")
    mk_d = nc.dram_tensor("mk", [128, 4 * IB], f32, kind="ExternalInput")
    yT_d = nc.dram_tensor("yT", [D, T], f32, kind="ExternalOutput")

    def mm(out, lhsT, rhs, start, stop):
        nc.tensor.matmul(out, lhsT, rhs, start=start, stop=stop)

    with tile.TileContext(nc) as tc:
        with tc.tile_pool(name="persist", bufs=1) as pp:
            wq_s = pp.tile([128, DC, JQ], f32r, tag="wq")
            wk_s = pp.tile([128, DC, JQ], f32r, tag="wk")
            wv_s = pp.tile([128, DC, JQ], f32r, tag="wv")
            wo_s = pp.tile([128, JCH, D], f32r, tag="wo")
            bq_s = pp.tile([128, JCH], f32, tag="bq")
            mk_s = pp.tile([128, 4, IB], f32, tag="mk")
            ones_s = pp.tile([1, 64], f32r, tag="ones")
            ones32_s = pp.tile([128, 64], f32, tag="ones32")
            qT_s = pp.tile([128, JCH, T], f32r, tag="qT")
            kT_s = pp.tile([128, JCH, T], f32r, tag="kT")
            v_s = pp.tile([128, NB, HEADS * VW], f32r, tag="v")
            on_s = pp.tile([128, JCH, T], f32r, tag="otn")

            nc.sync.dma_start(wq_s[:], wq_d.rearrange("(c p) j -> p c j", p=128))
            nc.sync.dma_start(wk_s[:], wk_d.rearrange("(c p) j -> p c j", p=128))
            nc.sync.dma_start(wv_s[:], wv_d.rearrange("(c p) j -> p c j", p=128))
            nc.sync.dma_start(wo_s[:], wo_d.rearrange("(c p) e -> p c e", p=128))
            nc.sync.dma_start(bq_s[:], bq_d[:])
            nc.sync.dma_start(mk_s[:], mk_d.rearrange("p (k i) -> p k i", k=4))
            nc.vector.memset(ones32_s[:], 1.0)
            nc.vector.tensor_copy(ones_s[:], ones32_s[0:1, :])

            import contextlib
            loop_cm = (tc.For_i(0, repeat, 1) if repeat > 1
                       else contextlib.nullcontext())
            with loop_cm:
                # ---------------- phase 1: qkv projection ----------------
                with tc.tile_pool(name="xs", bufs=2) as xp, \
                     tc.tile_pool(name="ps1", bufs=6, space="PSUM") as ps1:
                    for tci in range(TC):
                        x_t = xp.tile([128, DC, IB], f32r, tag="x")
                        nc.sync.dma_start(
                            x_t[:],
                            xT_d[:, ts(tci, IB)].rearrange("(c p) t -> p c t", p=128))
                        for jc in range(JCH):
                            q_ps = ps1.tile([128, IB], f32, tag="ps1")
                            for dc in range(DC):
                                mm(q_ps[:], wq_s[:, dc, ts(jc, 128)], x_t[:, dc, :],
                                   dc == 0, dc == DC - 1)
                            nc.vector.tensor_scalar_add(
                                qT_s[:, jc, ts(tci, IB)], q_ps[:], bq_s[:, jc:jc + 1])
                        for jc in range(JCH):
                            k_ps = ps1.tile([128, IB], f32, tag="ps1")
                            for dc in range(DC):
                                mm(k_ps[:], wk_s[:, dc, ts(jc, 128)], x_t[:, dc, :],
                                   dc == 0, dc == DC - 1)
                            nc.vector.tensor_copy(kT_s[:, jc, ts(tci, IB)], k_ps[:])
                        for s in range(IB // JB):
                            v_ps = ps1.tile([128, IB], f32, tag="ps1")
                            for dc in range(DC):
                                mm(v_ps[:, :JQ], x_t[:, dc, ts(s, 128)], wv_s[:, dc, :],
                                   dc == 0, dc == DC - 1)
                            bi = tci * (IB // JB) + s
                            vv = v_s[:, bi, :].rearrange("p (h c) -> p h c", c=VW)
                            nc.vector.tensor_copy(
                                vv[:, :, 0:HD],
                                v_ps[:, :JQ].rearrange("p (h c) -> p h c", c=HD))
                            nc.vector.tensor_copy(
                                vv[:, :, HD:VW],
                                ones32_s[:, 0:HEADS].rearrange(
                                    "p (a b) -> p a b", b=1))

                # ---------------- phase 2: causal attention ----------------
                with tc.tile_pool(name="pt", bufs=6) as ptp, \
                     tc.tile_pool(name="rsb", bufs=2) as rsb, \
                     tc.tile_pool(name="pss", bufs=4, space="PSUM") as pss, \
                     tc.tile_pool(name="pso", bufs=2, space="PSUM") as pso, \
                     tc.tile_pool(name="psr", bufs=2, space="PSUM") as psr:
                    for h in range(HEADS):
                        jc = h // 2
                        p0 = 64 * (h % 2)
                        for g in range(GB):
                            ncj = (g + 1) * (IB // JB)
                            o_ps = pso.tile([HD + 1, IB], f32, tag="ot")
                            for cj in range(ncj):
                                s_ps = pss.tile([128, IB], f32, tag="s")
                                mm(s_ps[:],
                                   kT_s[p0:p0 + HD, jc, ts(cj, JB)],
                                   qT_s[p0:p0 + HD, jc, ts(g, IB)],
                                   True, True)
                                p_t = ptp.tile([128, IB], f32r, tag="p")
                                nc.scalar.activation(
                                    p_t[:], s_ps[:],
                                    mybir.ActivationFunctionType.Exp, scale=0.125)
                                kd = cj - g * (IB // JB)
                                if kd >= 0:
                                    nc.vector.tensor_mul(p_t[:], p_t[:], mk_s[:, kd, :])
                                mm(o_ps[:],
                                   v_s[:, cj, :].rearrange(
                                       "p (h c) -> p h c", c=VW)[:, h, :],
                                   p_t[:], cj == 0, cj == ncj - 1)
                            # normalize columns by 1/l (row HD of o_ps)
                            r_t = rsb.tile([1, IB], f32r, tag="r")
                            with nc.allow_low_precision(reason="1/l in f32r feeds PE broadcast"):
                                nc.vector.reciprocal(r_t[:], o_ps[HD:HD + 1, :])
                            rb_ps = psr.tile([64, IB], f32, tag="rb")
                            mm(rb_ps[:], ones_s[:], r_t[:], True, True)
                            rb_t = rsb.tile([64, IB], f32, tag="rbs")
                            nc.vector.tensor_copy(rb_t[:], rb_ps[:])
                            nc.vector.tensor_mul(
                                on_s[p0:p0 + HD, jc, ts(g, IB)],
                                o_ps[0:HD, :], rb_t[:])

                # ---------------- phase 3: output projection ----------------
                with tc.tile_pool(name="ys", bufs=4) as ysp, \
                     tc.tile_pool(name="ps3", bufs=4, space="PSUM") as ps3:
                    for ec in range(DC):
                        for tci in range(TC):
                            y_ps = ps3.tile([128, IB], f32, tag="y")
                            for cc in range(JCH):
                                mm(y_ps[:], wo_s[:, cc, ts(ec, 128)],
                                   on_s[:, cc, ts(tci, IB)],
                                   cc == 0, cc == JCH - 1)
                            y_sb = ysp.tile([128, IB], f32, tag="ysb")
                            nc.vector.tensor_copy(y_sb[:], y_ps[:])
                            nc.sync.dma_start(
                                yT_d[ts(ec, 128), ts(tci, IB)], y_sb[:])

    _split_sync_waits(nc, max_waits=1)
    return nc


def _get_program(T, repeat=1):
    key = (T, repeat)
    if key not in _PROG_CACHE:
        _PROG_CACHE[key] = _build_program(T, repeat)
    return _PROG_CACHE[key]


def _make_masks():
    ii = np.arange(IB)[None, :]
    jj = np.arange(JB)[:, None]
    return np.concatenate(
        [(ii >= jj + JB * k).astype(np.float32) for k in range(4)],
        axis=1)  # [128, 4*IB]


def _core_inputs(x, qkv_w, qkv_b, out_w, T):
    D = D_MODEL
    masks = _make_masks()
    maps = []
    for core in range(N_CORES):
        b, g = divmod(core, N_GROUPS)
        r0 = JQ * g
        xT = np.ascontiguousarray(x[b].T)                       # [D, T]
        wq = np.ascontiguousarray(qkv_w[r0:r0 + JQ, :].T)       # [D, JQ]
        wk = np.ascontiguousarray(qkv_w[D + r0:D + r0 + JQ, :].T)
        wv = np.ascontiguousarray(qkv_w[2 * D + r0:2 * D + r0 + JQ, :].T)
        wo = np.ascontiguousarray(out_w[:, r0:r0 + JQ].T)       # [JQ, D]
        bq = np.ascontiguousarray(
            qkv_b[r0:r0 + JQ].reshape(JQ // 128, 128).T)        # [128, JCH]
        maps.append({"xT": xT, "wq": wq, "wk": wk, "wv": wv,
                     "wo": wo, "bq": bq, "mk": masks})
    return maps


def _gather(results, out_b, out_w, qkv_b, B, T):
    D = D_MODEL
    y = np.zeros((B, T, D), np.float32)
    for core, om in enumerate(results):
        b = core // N_GROUPS
        y[b] += om["yT"].T
    bv = qkv_b[2 * D:3 * D]
    y += (out_b + out_w @ bv)[None, None, :].astype(np.float32)
    return y


def kernel(x, qkv_w, qkv_b, out_w, out_b):
    from concourse.bass_utils import run_bass_kernel_spmd

    x = np.asarray(x, np.float32)
    B, T, D = x.shape
    assert D == D_MODEL and B * N_GROUPS == N_CORES
    nc = _get_program(T)
    in_maps = _core_inputs(x, qkv_w, qkv_b, out_w, T)
    res = run_bass_kernel_spmd(nc, in_maps, list(range(N_CORES)))
    return _gather(res.results, np.asarray(out_b, np.float32),
                   np.asarray(out_w, np.float32),
                   np.asarray(qkv_b, np.float32), B, T)


# revision 14
# speedup vs baseline: 157.3618x; 1.0331x over previous
"""Multi-head causal attention (B=2, T=2048, D=1024, H=16) on 8 trn2 cores.

Sharding: core = (b, g) with b = batch (2), g = head-group of 4 heads (4).
Each core:
  phase 1: qkv projection for its 4 heads  (qT/kT feature-major, v token-major)
  phase 2: causal attention, scores computed transposed (S^T[j, i]) so the
           softmax denominator comes out of the PV matmul via an appended
           ones-column on V, and no on-device transposes are needed
  phase 3: partial output projection y^T = wo_slice^T @ attn^T
Host: sums the 4 per-batch partials, adds out_b + out_w @ v_bias (v-bias
commutes through the normalized softmax), k-bias dropped entirely (softmax
row-shift invariance), q-bias applied on device.
"""

import sys

if "/opt/trn_rl_repo" not in sys.path:
    sys.path.insert(0, "/opt/trn_rl_repo")

import numpy as np

D_MODEL = 1024
N_HEADS = 16
HEAD_DIM = 64
N_CORES = 8
N_GROUPS = 4                     # head groups (tensor parallel)
HEADS = N_HEADS // N_GROUPS      # heads per core = 4
JQ = HEADS * HEAD_DIM            # per-core q/k/v features = 256
IB = 512                         # i-block (query block) width
JB = 128                         # j-chunk (key block) width


def _split_sync_waits(nc, max_waits=1):
    """walrus in this env rejects >1 sync wait on an instruction; move
    extras onto preceding same-engine NOPs."""
    import bass_rust
    import concourse.mybir as mybir

    uid = 0
    for f in nc.m.functions:
        for bb in f.blocks:
            insts = list(bb.instructions)
            out = []
            changed = False
            for ins in insts:
                si = ins.sync_info
                waits = list(si.on_wait) if si is not None else []
                if len(waits) > max_waits:
                    changed = True
                    extras, keep = waits[:-max_waits], waits[-max_waits:]
                    for i in range(0, len(extras), max_waits):
                        grp = extras[i:i + max_waits]
                        nop = mybir.InstNoOp(name=f"WS-{uid}", ins=[], outs=[])
                        uid += 1
                        nop.engine = ins.engine
                        nop.sync_info = bass_rust.SyncInfo(on_wait=grp, on_update=[])
                        out.append(nop)
                    ins.sync_info = bass_rust.SyncInfo(
                        on_wait=keep, on_update=list(si.on_update))
                out.append(ins)
            if changed:
                bb.instructions = out
    return nc


_PROG_CACHE = {}


def _build_program(T, repeat=1):
    import concourse.bass as bass
    import concourse.mybir as mybir
    import concourse.tile as tile
    from concourse.bass import ts

    f32 = mybir.dt.float32
    f32r = mybir.dt.float32r
    D = D_MODEL
    HD = HEAD_DIM
    DC = D // 128                 # contraction chunks (8)
    JCH = JQ // 128               # q/k feature chunks (2)
    TC = T // IB                  # 512-wide token chunks
    NB = T // JB                  # 128-wide token chunks
    GB = T // IB                  # i-blocks
    VW = HD + 1                   # v + ones column

    nc = bass.Bass(target_bir_lowering=False)

    xT_d = nc.dram_tensor("xT", [D, T], f32r, kind="ExternalInput")
    wq_d = nc.dram_tensor("wq", [D, JQ], f32r, kind="ExternalInput")
    wk_d = nc.dram_tensor("wk", [D, JQ], f32r, kind="ExternalInput")
    wv_d = nc.dram_tensor("wv", [D, JQ], f32r, kind="ExternalInput")
    wo_d = nc.dram_tensor("wo", [JQ, D], f32r, kind="ExternalInput")
    bq_d = nc.dram_tensor("bq", [128, JCH], f32, kind="ExternalInput")
    mk_d = nc.dram_tensor("mk", [128, 4 * IB], f32, kind="ExternalInput")
    yT_d = nc.dram_tensor("yT", [D, T], f32, kind="ExternalOutput")

    def mm(out, lhsT, rhs, start, stop):
        nc.tensor.matmul(out, lhsT, rhs, start=start, stop=stop)

    with tile.TileContext(nc) as tc:
        with tc.tile_pool(name="persist", bufs=1) as pp:
            wq_s = pp.tile([128, DC, JQ], f32r, tag="wq")
            wk_s = pp.tile([128, DC, JQ], f32r, tag="wk")
            wv_s = pp.tile([128, DC, JQ], f32r, tag="wv")
            wo_s = pp.tile([128, JCH, D], f32r, tag="wo")
            bq_s = pp.tile([128, JCH], f32, tag="bq")
            mk_s = pp.tile([128, 4, IB], f32, tag="mk")
            ones_s = pp.tile([1, 64], f32r, tag="ones")
            ones32_s = pp.tile([128, 64], f32, tag="ones32")
            qT_s = pp.tile([128, JCH, T], f32r, tag="qT")
            kT_s = pp.tile([128, JCH, T], f32r, tag="kT")
            v_s = pp.tile([128, NB, HEADS * VW], f32r, tag="v")
            on_s = pp.tile([128, JCH, T], f32r, tag="otn")

            nc.sync.dma_start(wq_s[:], wq_d.rearrange("(c p) j -> p c j", p=128))
            nc.sync.dma_start(bq_s[:], bq_d[:])

            import contextlib
            xp_cm = tc.tile_pool(name="xs", bufs=2)
            xp = xp_cm.__enter__()
            x_first = None
            if repeat == 1:
                # preload first x chunk ahead of the bulk weight DMAs so the
                # first q-matmuls start as early as possible
                x_first = xp.tile([128, DC, IB], f32r, tag="x")
                nc.sync.dma_start(
                    x_first[:],
                    xT_d[:, ts(0, IB)].rearrange("(c p) t -> p c t", p=128))
            nc.sync.dma_start(wk_s[:], wk_d.rearrange("(c p) j -> p c j", p=128))
            nc.sync.dma_start(wv_s[:], wv_d.rearrange("(c p) j -> p c j", p=128))
            nc.sync.dma_start(mk_s[:], mk_d.rearrange("p (k i) -> p k i", k=4))
            nc.sync.dma_start(wo_s[:], wo_d.rearrange("(c p) e -> p c e", p=128))
            nc.vector.memset(ones32_s[:], 1.0)
            nc.vector.tensor_copy(ones_s[:], ones32_s[0:1, :])

            loop_cm = (tc.For_i(0, repeat, 1) if repeat > 1
                       else contextlib.nullcontext())
            with loop_cm:
                # ---------------- phase 1: qkv projection ----------------
                with tc.tile_pool(name="ps1", bufs=6, space="PSUM") as ps1:
                    for tci in range(TC):
                        if tci == 0 and x_first is not None:
                            x_t = x_first
                        else:
                            x_t = xp.tile([128, DC, IB], f32r, tag="x")
                            nc.sync.dma_start(
                                x_t[:],
                                xT_d[:, ts(tci, IB)].rearrange(
                                    "(c p) t -> p c t", p=128))
                        for jc in range(JCH):
                            q_ps = ps1.tile([128, IB], f32, tag="ps1")
                            for dc in range(DC):
                                mm(q_ps[:], wq_s[:, dc, ts(jc, 128)], x_t[:, dc, :],
                                   dc == 0, dc == DC - 1)
                            nc.vector.tensor_scalar_add(
                                qT_s[:, jc, ts(tci, IB)], q_ps[:], bq_s[:, jc:jc + 1])
                        for jc in range(JCH):
                            k_ps = ps1.tile([128, IB], f32, tag="ps1")
                            for dc in range(DC):
                                mm(k_ps[:], wk_s[:, dc, ts(jc, 128)], x_t[:, dc, :],
                                   dc == 0, dc == DC - 1)
                            nc.vector.tensor_copy(kT_s[:, jc, ts(tci, IB)], k_ps[:])
                        for s in range(IB // JB):
                            v_ps = ps1.tile([128, IB], f32, tag="ps1")
                            for dc in range(DC):
                                mm(v_ps[:, :JQ], x_t[:, dc, ts(s, 128)], wv_s[:, dc, :],
                                   dc == 0, dc == DC - 1)
                            bi = tci * (IB // JB) + s
                            vv = v_s[:, bi, :].rearrange("p (h c) -> p h c", c=VW)
                            nc.vector.tensor_copy(
                                vv[:, :, 0:HD],
                                v_ps[:, :JQ].rearrange("p (h c) -> p h c", c=HD))
                            nc.vector.tensor_copy(
                                vv[:, :, HD:VW],
                                ones32_s[:, 0:HEADS].rearrange(
                                    "p (a b) -> p a b", b=1))

                # -------- phases 2+3: attention, out-proj interleaved per g --------
                with tc.tile_pool(name="pt", bufs=6) as ptp, \
                     tc.tile_pool(name="rsb", bufs=3) as rsb, \
                     tc.tile_pool(name="ys", bufs=4) as ysp, \
                     tc.tile_pool(name="pss", bufs=2, space="PSUM") as pss, \
                     tc.tile_pool(name="pso", bufs=2, space="PSUM") as pso, \
                     tc.tile_pool(name="ps3", bufs=2, space="PSUM") as ps3:
                    for g in range(GB):
                        ncj = (g + 1) * (IB // JB)
                        for jc in range(JCH):
                            # heads 2*jc (PE rows 0-63) and 2*jc+1 (rows 64-127)
                            # interleaved: adjacent S-MMs use disjoint row
                            # groups and run concurrently in the array.
                            # j-chunks processed in pairs: one 2-bank PSUM
                            # tile per pair -> one exp + one mask-mul per pair.
                            o_ps = [pso.tile([HD + 1, IB], f32, tag="ot",
                                             name=f"ot{e}")
                                    for e in range(2)]
                            for cj2 in range(0, ncj, 2):
                                kd = cj2 - g * (IB // JB)
                                p_ts = []
                                for e in range(2):
                                    p0 = 64 * e
                                    s_ps = pss.tile([128, 2 * IB], f32, tag="s")
                                    for u in range(2):
                                        mm(s_ps[:, u * IB:(u + 1) * IB],
                                           kT_s[p0:p0 + HD, jc, ts(cj2 + u, JB)],
                                           qT_s[p0:p0 + HD, jc, ts(g, IB)],
                                           True, True)
                                    p_t = ptp.tile([128, 2 * IB], f32r, tag="p")
                                    nc.scalar.activation(
                                        p_t[:], s_ps[:],
                                        mybir.ActivationFunctionType.Exp,
                                        scale=0.125)
                                    if kd >= 0:
                                        nc.vector.tensor_mul(
                                            p_t[:], p_t[:],
                                            mk_s[:, kd:kd + 2, :].rearrange(
                                                "p k i -> p (k i)"))
                                    p_ts.append(p_t)
                                for e in range(2):
                                    h = 2 * jc + e
                                    for u in range(2):
                                        cj = cj2 + u
                                        mm(o_ps[e][:],
                                           v_s[:, cj, :].rearrange(
                                               "p (h c) -> p h c", c=VW)[:, h, :],
                                           p_ts[e][:, u * IB:(u + 1) * IB],
                                           cj == 0, cj == ncj - 1)
                            for e in range(2):
                                p0 = 64 * e
                                # normalize columns by 1/l (row HD of o_ps)
                                r_t = rsb.tile([1, IB], f32r, tag="r")
                                with nc.allow_low_precision(reason="1/l feeds f32r matmul"):
                                    nc.vector.reciprocal(
                                        r_t[:], o_ps[e][HD:HD + 1, :])
                                rb_ps = ps3.tile([128, IB], f32, tag="y")
                                mm(rb_ps[0:64, :], ones_s[:], r_t[:], True, True)
                                rb_t = rsb.tile([64, IB], f32, tag="rbs")
                                nc.vector.tensor_copy(rb_t[:], rb_ps[0:64, :])
                                nc.vector.tensor_mul(
                                    on_s[p0:p0 + HD, jc, ts(g, IB)],
                                    o_ps[e][0:HD, :], rb_t[:])
                        # out-proj for the finished i-block g overlaps later g's
                        for ec in range(DC):
                            y_ps = ps3.tile([128, IB], f32, tag="y")
                            for cc in range(JCH):
                                mm(y_ps[:], wo_s[:, cc, ts(ec, 128)],
                                   on_s[:, cc, ts(g, IB)],
                                   cc == 0, cc == JCH - 1)
                            y_sb = ysp.tile([128, IB], f32, tag="ysb")
                            nc.vector.tensor_copy(y_sb[:], y_ps[:])
                            nc.sync.dma_start(
                                yT_d[ts(ec, 128), ts(g, IB)], y_sb[:])

            xp_cm.__exit__(None, None, None)

    _split_sync_waits(nc, max_waits=1)
    return nc


def _get_program(T, repeat=1):
    key = (T, repeat)
    if key not in _PROG_CACHE:
        _PROG_CACHE[key] = _build_program(T, repeat)
    return _PROG_CACHE[key]


def _make_masks():
    ii = np.arange(IB)[None, :]
    jj = np.arange(JB)[:, None]
    return np.concatenate(
        [(ii >= jj + JB * k).astype(np.float32) for k in range(4)],
        axis=1)  # [128, 4*IB]


def _core_inputs(x, qkv_w, qkv_b, out_w, T):
    D = D_MODEL
    masks = _make_masks()
    maps = []
    for core in range(N_CORES):
        b, g = divmod(core, N_GROUPS)
        r0 = JQ * g
        xT = np.ascontiguousarray(x[b].T)                       # [D, T]
        wq = np.ascontiguousarray(qkv_w[r0:r0 + JQ, :].T)       # [D, JQ]
        wk = np.ascontiguousarray(qkv_w[D + r0:D + r0 + JQ, :].T)
        wv = np.ascontiguousarray(qkv_w[2 * D + r0:2 * D + r0 + JQ, :].T)
        wo = np.ascontiguousarray(out_w[:, r0:r0 + JQ].T)       # [JQ, D]
        bq = np.ascontiguousarray(
            qkv_b[r0:r0 + JQ].reshape(JQ // 128, 128).T)        # [128, JCH]
        maps.append({"xT": xT, "wq": wq, "wk": wk, "wv": wv,
                     "wo": wo, "bq": bq, "mk": masks})
    return maps


def _gather(results, out_b, out_w, qkv_b, B, T):
    D = D_MODEL
    y = np.zeros((B, T, D), np.float32)
    for core, om in enumerate(results):
        b = core // N_GROUPS
        y[b] += om["yT"].T
    bv = qkv_b[2 * D:3 * D]
    y += (out_b + out_w @ bv)[None, None, :].astype(np.float32)
    return y


def kernel(x, qkv_w, qkv_b, out_w, out_b):
    from concourse.bass_utils import run_bass_kernel_spmd

    x = np.asarray(x, np.float32)
    B, T, D = x.shape
    assert D == D_MODEL and B * N_GROUPS == N_CORES
    nc = _get_program(T)
    in_maps = _core_inputs(x, qkv_w, qkv_b, out_w, T)
    res = run_bass_kernel_spmd(nc, in_maps, list(range(N_CORES)))
    return _gather(res.results, np.asarray(out_b, np.float32),
                   np.asarray(out_w, np.float32),
                   np.asarray(qkv_b, np.float32), B, T)
